# revision 1
# baseline (speedup 1.0000x reference)
"""Multi-head attention (nn_AttentionMechanism) on 8 Trainium2 NeuronCores.

Reference computation (per batch n):
    v = values @ Wv.T ; k = keys @ Wk.T ; q = query @ Wq.T   (all [S, D])
    energy[h,i,j] = sum_d q[i,h,d] k[j,h,d]
    attn = softmax(energy / sqrt(D), axis=j)
    out = (attn @ v per head, concat heads) @ Wo.T + bo

Sharding: data-parallel over (batch, seq-half): core c handles batch c//2,
query rows (c%2)*1024..+1024. K/V are computed for the full 2048-row sequence
on both cores of a pair (duplicated compute, zero collectives).

On-chip strategy (per core):
 - Matmul operands in fp16 (1 cycle/row on the PE + fast weight load);
   accumulation in fp32 PSUM. Inputs are PE-transposed in fp32, cast to fp16
   on the PSUM->SBUF copy.
 - Projections q/k produce TRANSPOSED outputs (head-dim on partitions).
   Energy is computed transposed ([k-part, q-free]) so the softmax
   denominator rides along as a ones-column in the attn@v matmul.
 - k/v projections run per head-pair INSIDE the attention loop so their PE
   work overlaps the (bottleneck) ScalarE exp stream; Wo transposes are
   likewise spread across the attention pairs.
 - Heads processed in pairs: the two K=64 energy matmuls occupy different
   row-groups of the PE array and run concurrently; their exps are fused
   into one 1024-wide ACTIVATE.
 - Softmax without max-subtraction (energy/32 is ~N(0, 0.25); exp never
   overflows for this problem's input distribution).
"""

import numpy as np

import concourse.bass as bass
import concourse.mybir as mybir
import concourse.tile as tile
from concourse import bacc
from concourse.bass_utils import run_bass_kernel_spmd

F32 = mybir.dt.float32
F16 = mybir.dt.float16
AF = mybir.ActivationFunctionType
ALU = mybir.AluOpType

P = 128
D = 1024
H = 16
DH = 64
NQ = 1024  # q rows per core
NK = 2048  # kv rows per core
SCALE = 1.0 / 32.0  # 1/sqrt(D)

_CACHE = {}


def build():
    nc = bacc.Bacc("TRN2", target_bir_lowering=False, debug=False)

    xq = nc.dram_tensor("xq", [NQ, D], F32, kind="ExternalInput")
    xk = nc.dram_tensor("xk", [NK, D], F32, kind="ExternalInput")
    xv = nc.dram_tensor("xv", [NK, D], F32, kind="ExternalInput")
    wq = nc.dram_tensor("wq", [D, D], F32, kind="ExternalInput")
    wk = nc.dram_tensor("wk", [D, D], F32, kind="ExternalInput")
    wv = nc.dram_tensor("wv", [D, D], F32, kind="ExternalInput")
    wo = nc.dram_tensor("wo", [D, D], F32, kind="ExternalInput")
    bo = nc.dram_tensor("bo", [1, D], F32, kind="ExternalInput")
    ident_d = nc.dram_tensor("ident", [P, P], F32, kind="ExternalInput")
    ones_d = nc.dram_tensor("ones", [P, 2 * H], F16, kind="ExternalInput")
    out = nc.dram_tensor("out", [NQ, D], F32, kind="ExternalOutput")

    with tile.TileContext(nc) as tc:
        with (
            tc.tile_pool(name="consts", bufs=1) as consts,
            tc.tile_pool(name="glob", bufs=1) as glob,
        ):
            ident = consts.tile([P, P], F32)
            nc.sync.dma_start(ident[:], ident_d[:])
            ident16 = consts.tile([P, P], F16, name="ident16")
            nc.vector.tensor_copy(ident16[:], ident[:])

            qT = glob.tile([P, 8, NQ], F16, name="qT")      # 16 KB/part
            catT = glob.tile([P, 8, NQ], F16, name="catT")  # 16 KB/part

            with (
                tc.tile_pool(name="bglob", bufs=1) as bglob,
                tc.tile_pool(name="wtp", bufs=1) as wtp,
            ):
                xkT = bglob.tile([P, 8, NK], F16, name="xkT")   # 32 KB
                xvT = bglob.tile([P, 8, NK], F16, name="xvT")   # 32 KB
                wkT = wtp.tile([P, 8, D], F16, name="wkT")      # 16 KB
                wvT = wtp.tile([P, 8, D], F16, name="wvT")      # 16 KB

                # ---------------- Phase A ----------------
                with (
                    tc.tile_pool(name="xin", bufs=2) as xinp,
                    tc.tile_pool(name="wta", bufs=1) as wtap,
                    tc.tile_pool(name="psA", bufs=1, space="PSUM") as psA,
                ):
                    # alternate engines on the psum->sbuf cast copies
                    _eng = [0]

                    def _copy(dst, src):
                        if _eng[0] % 2 == 0:
                            nc.vector.tensor_copy(dst, src)
                        else:
                            nc.scalar.copy(dst, src)
                        _eng[0] += 1

                    def build_wT(w_dram, wT, wname):
                        """wT[:, dc, oc*128:+128] = W[oc-chunk, dc-chunk].T"""
                        for oc in range(8):
                            wnat = xinp.tile(
                                [P, D], F32, tag="xnat", bufs=3, name=f"{wname}_n{oc}"
                            )
                            nc.sync.dma_start(
                                wnat[:], w_dram[oc * P : (oc + 1) * P, :]
                            )
                            for dcq in range(2):
                                ps = psA.tile(
                                    [P, 512], F32, tag="tps", bufs=4,
                                    name=f"{wname}_t{oc}_{dcq}",
                                )
                                for j in range(4):
                                    nc.tensor.transpose(
                                        ps[:, j * P : (j + 1) * P],
                                        wnat[
                                            :,
                                            (dcq * 4 + j) * P : (dcq * 4 + j + 1) * P,
                                        ],
                                        ident[:],
                                    )
                                _copy(
                                    wT[
                                        :,
                                        dcq * 4 : (dcq + 1) * 4,
                                        oc * P : (oc + 1) * P,
                                    ],
                                    ps[:].rearrange("p (j c) -> p j c", c=P),
                                )

                    def build_xT(x_dram, xT, ib, xname):
                        """xT[:, dc, ib*512 ...] = x[i-block ib].T (fp16)"""
                        xnat = xinp.tile(
                            [P, 4, D], F32, tag="xbig", name=f"{xname}_n{ib}"
                        )
                        nc.sync.dma_start(
                            xnat[:],
                            x_dram[ib * 512 : (ib + 1) * 512, :].rearrange(
                                "(s p) d -> p s d", p=P
                            ),
                        )
                        for dc in range(8):
                            ps = psA.tile(
                                [P, 512], F32, tag="tps", bufs=4,
                                name=f"{xname}_t{ib}_{dc}",
                            )
                            for s in range(4):
                                nc.tensor.transpose(
                                    ps[:, s * P : (s + 1) * P],
                                    xnat[:, s, dc * P : (dc + 1) * P],
                                    ident[:],
                                )
                            _copy(xT[:, dc, ib * 512 : (ib + 1) * 512], ps[:])

                    # q projection (all of it) + k/v input transposes
                    wqT = wtap.tile([P, 8, D], F16, tag="wt", name="wqT")
                    build_wT(wq, wqT, "wqT")
                    xqT = wtap.tile([P, 8, NQ], F16, tag="xqT", name="xqT")
                    for ib in range(2):
                        build_xT(xq, xqT, ib, "xqT")
                    # build k/v weight transposes BEFORE qproj halves:
                    # each qproj half fills the PE while the next W/x DMAs
                    # prefetch ahead
                    build_wT(wk, wkT, "wkT")
                    for ib in range(1):
                        for oc in range(8):
                            qps = psA.tile(
                                [P, 512], F32, tag="qps", bufs=2,
                                name=f"qpsA{ib}_{oc}",
                            )
                            for dc in range(8):
                                nc.tensor.matmul(
                                    qps[:],
                                    wqT[:, dc, oc * P : (oc + 1) * P],
                                    xqT[:, dc, ib * 512 : (ib + 1) * 512],
                                    start=(dc == 0),
                                    stop=(dc == 7),
                                )
                            nc.scalar.copy(
                                qT[:, oc, ib * 512 : (ib + 1) * 512], qps[:]
                            )
                    build_wT(wv, wvT, "wvT")
                    for ib in range(1, 2):
                        for oc in range(8):
                            qps = psA.tile(
                                [P, 512], F32, tag="qps", bufs=2,
                                name=f"qps{ib}_{oc}",
                            )
                            for dc in range(8):
                                nc.tensor.matmul(
                                    qps[:],
                                    wqT[:, dc, oc * P : (oc + 1) * P],
                                    xqT[:, dc, ib * 512 : (ib + 1) * 512],
                                    start=(dc == 0),
                                    stop=(dc == 7),
                                )
                            nc.scalar.copy(
                                qT[:, oc, ib * 512 : (ib + 1) * 512], qps[:]
                            )

                    for ib in range(4):
                        build_xT(xk, xkT, ib, "xkT")
                    for ib in range(4):
                        build_xT(xv, xvT, ib, "xvT")

                # woT lives from here (reuses phase-A space) through phase C
                with tc.tile_pool(name="wop", bufs=1) as wop:
                    woT = wop.tile([P, 8, D], F16, name="woT")  # 16 KB

                    # ---------------- Phase B ----------------
                    with (
                        tc.tile_pool(name="kv", bufs=2) as kvp,
                        tc.tile_pool(name="pp", bufs=4) as ppp,
                        tc.tile_pool(name="dd", bufs=3) as ddp,
                        tc.tile_pool(name="psB", bufs=1, space="PSUM") as psB,
                    ):

                        def make_preamble(c):
                            """Allocate pair-c tiles; return (kT, vx, steps).

                            Each step is a thunk emitting one chunk of the
                            k/v projection (plus Wo transposes) so it can be
                            interleaved into the previous pair's attention.
                            """
                            kT = kvp.tile([P, NK], F16, tag="kt", name=f"kT{c}")
                            vT = kvp.tile([P, NK], F16, tag="vt", name=f"vT{c}")
                            vx = kvp.tile(
                                [P, 16, 2, 65], F16, tag="vx", name=f"vx{c}"
                            )
                            steps = []

                            def ones_step():
                                nc.sync.dma_start(
                                    vx[:, :, :, 64:65],
                                    ones_d[:, :, None].rearrange(
                                        "p (kc t) u -> p kc t u", t=2
                                    ),
                                )

                            steps.append(ones_step)

                            def proj_step(wT, xT, dst, ic4, nm):
                                def _f():
                                    ps_ = psB.tile(
                                        [P, 512], F32, tag="kvps", bufs=2,
                                        name=f"{nm}{c}_{ic4}",
                                    )
                                    for dc in range(8):
                                        nc.tensor.matmul(
                                            ps_[:],
                                            wT[:, dc, c * P : (c + 1) * P],
                                            xT[:, dc, ic4 * 512 : (ic4 + 1) * 512],
                                            start=(dc == 0),
                                            stop=(dc == 7),
                                        )
                                    nc.vector.tensor_copy(
                                        dst[:, ic4 * 512 : (ic4 + 1) * 512],
                                        ps_[:],
                                    )

                                return _f

                            for ic4 in range(4):
                                steps.append(proj_step(wkT, xkT, kT, ic4, "kps"))
                            for ic4 in range(4):
                                steps.append(proj_step(wvT, xvT, vT, ic4, "vps"))

                            def vt_step(kc16):
                                def _f():
                                    for k2 in (kc16, kc16 + 1):
                                        tvp = psB.tile(
                                            [P, P], F16, tag="kvps", bufs=2,
                                            name=f"tvp{c}_{k2}",
                                        )
                                        nc.tensor.transpose(
                                            tvp[:],
                                            vT[:, k2 * P : (k2 + 1) * P],
                                            ident16[:],
                                        )
                                        nc.vector.tensor_copy(
                                            vx[:, k2, :, 0:64],
                                            tvp[:].rearrange(
                                                "p (t c) -> p t c", c=64
                                            ),
                                        )

                                return _f

                            for kc16 in range(0, 16, 2):
                                steps.append(vt_step(kc16))

                            # spread the Wo transpose-build over pairs 2..5
                            if 2 <= c <= 5:
                                def wo_step(oc):
                                    def _f():
                                        wnat = kvp.tile(
                                            [P, D], F32, tag="vt",
                                            name=f"woT_n{oc}",
                                        )
                                        nc.sync.dma_start(
                                            wnat[:],
                                            wo[oc * P : (oc + 1) * P, :],
                                        )
                                        for dcq in range(2):
                                            pw = psB.tile(
                                                [P, 512], F32, tag="kvps",
                                                bufs=2,
                                                name=f"woT_t{oc}_{dcq}",
                                            )
                                            for j in range(4):
                                                nc.tensor.transpose(
                                                    pw[:, j * P : (j + 1) * P],
                                                    wnat[
                                                        :,
                                                        (dcq * 4 + j) * P : (dcq * 4 + j + 1) * P,
                                                    ],
                                                    ident[:],
                                                )
                                            nc.vector.tensor_copy(
                                                woT[
                                                    :,
                                                    dcq * 4 : (dcq + 1) * 4,
                                                    oc * P : (oc + 1) * P,
                                                ],
                                                pw[:].rearrange(
                                                    "p (j c) -> p j c", c=P
                                                ),
                                            )

                                    return _f

                                for oc in (2 * (c - 2), 2 * (c - 2) + 1):
                                    steps.append(wo_step(oc))

                            return kT, vx, steps

                        # prologue: pair 0's projections run un-overlapped
                        kT, vx, steps = make_preamble(0)
                        for st in steps:
                            st()

                        for c in range(8):  # head pair
                            if c < 7:
                                kT_n, vx_n, steps = make_preamble(c + 1)
                            else:
                                kT_n, vx_n, steps = None, None, []
                            si = 0
                            for qt in range(2):
                                o0 = psB.tile(
                                    [65, 512], F32, tag="o0", bufs=1,
                                    name=f"o0_{c}_{qt}",
                                )
                                o1 = psB.tile(
                                    [65, 512], F32, tag="o1", bufs=1,
                                    name=f"o1_{c}_{qt}",
                                )
                                def energy(kc):
                                    ee = psB.tile(
                                        [P, 1024], F32, tag="ee", bufs=2,
                                        name=f"ee_{c}_{qt}_{kc}",
                                    )
                                    nc.tensor.matmul(
                                        ee[:, 0:512],
                                        kT[0:DH, kc * P : (kc + 1) * P],
                                        qT[0:DH, c, qt * 512 : (qt + 1) * 512],
                                        start=True,
                                        stop=True,
                                    )
                                    nc.tensor.matmul(
                                        ee[:, 512:1024],
                                        kT[DH:P, kc * P : (kc + 1) * P],
                                        qT[DH:P, c, qt * 512 : (qt + 1) * 512],
                                        start=True,
                                        stop=True,
                                    )
                                    pp = ppp.tile(
                                        [P, 1024], F16, tag="pp",
                                        name=f"pp_{c}_{qt}_{kc}",
                                    )
                                    nc.scalar.activation(
                                        pp[:], ee[:], AF.Exp, scale=SCALE
                                    )
                                    return pp

                                # energy runs one iteration ahead of attn@v
                                # so the in-order PE stream never stalls on
                                # the exp of the current iteration.
                                pp_cur = energy(0)
                                for kc in range(16):
                                    if kc < 15:
                                        pp_nxt = energy(kc + 1)
                                    nc.tensor.matmul(
                                        o0[:],
                                        vx[:, kc, 0, :],
                                        pp_cur[:, 0:512],
                                        start=(kc == 0),
                                        stop=(kc == 15),
                                    )
                                    nc.tensor.matmul(
                                        o1[:],
                                        vx[:, kc, 1, :],
                                        pp_cur[:, 512:1024],
                                        start=(kc == 0),
                                        stop=(kc == 15),
                                    )
                                    if kc < 15:
                                        pp_cur = pp_nxt
                                    # interleave one next-pair preamble step
                                    # every other iteration
                                    if kc % 2 == 1 and si < len(steps):
                                        steps[si]()
                                        si += 1
                                # normalize: catT[rows, c, qt] = o[0:64]/o[64]
                                for j, ops in enumerate((o0, o1)):
                                    stage = ddp.tile(
                                        [P, 512], F32, tag="stage",
                                        name=f"stage{c}_{qt}_{j}",
                                    )
                                    nc.vector.tensor_copy(
                                        stage[0:65, :], ops[0:65, :]
                                    )
                                    dsh = ddp.tile(
                                        [1, 512], F32, tag="dsh",
                                        name=f"dsh{c}_{qt}_{j}",
                                    )
                                    nc.sync.dma_start(
                                        dsh[0:1, :], stage[64:65, :]
                                    )
                                    rec = ddp.tile(
                                        [P, 512], F32, tag="rec",
                                        name=f"rec{c}_{qt}_{j}",
                                    )
                                    nc.vector.reciprocal_approx_fast(
                                        out=rec[0:1, :], in_=dsh[0:1, :]
                                    )
                                    bc = ddp.tile(
                                        [DH, 512], F32, tag="bc",
                                        name=f"bc{c}_{qt}_{j}",
                                    )
                                    nc.gpsimd.partition_broadcast(
                                        bc[:], rec[0:1, :]
                                    )
                                    if j == 0:
                                        nc.vector.tensor_tensor(
                                            catT[
                                                0:DH, c, qt * 512 : (qt + 1) * 512
                                            ],
                                            stage[0:DH, :],
                                            bc[:],
                                            ALU.mult,
                                        )
                                    else:
                                        stg = ddp.tile(
                                            [DH, 512], F16, tag="stg",
                                            name=f"stg{c}_{qt}",
                                        )
                                        nc.vector.tensor_tensor(
                                            stg[:], stage[0:DH, :], bc[:],
                                            ALU.mult,
                                        )
                                        nc.sync.dma_start(
                                            catT[
                                                DH:P, c, qt * 512 : (qt + 1) * 512
                                            ],
                                            stg[:],
                                        )
                            # any remaining preamble steps
                            while si < len(steps):
                                steps[si]()
                                si += 1
                            kT, vx = kT_n, vx_n

                    # ---------------- Phase C: output projection ----------
                    with (
                        tc.tile_pool(name="osb", bufs=3) as osbp,
                        tc.tile_pool(name="psC", bufs=1, space="PSUM") as psC,
                    ):
                        bo_st = osbp.tile([P, D], F32, tag="bo_st", name="bo_st")
                        nc.sync.dma_start(bo_st[0:1, :], bo[:])
                        bo_bc = osbp.tile([P, D], F32, tag="bo_bc", name="bo_bc")
                        nc.gpsimd.partition_broadcast(bo_bc[:], bo_st[0:1, :])

                        for ic in range(8):
                            ot = osbp.tile([P, D], F32, tag="ot", name=f"ot{ic}")
                            for oc2 in range(2):
                                ops_ = psC.tile(
                                    [P, 512], F32, tag="ops", bufs=2,
                                    name=f"ops{ic}_{oc2}",
                                )
                                for dc in range(8):
                                    nc.tensor.matmul(
                                        ops_[:],
                                        catT[:, dc, ic * P : (ic + 1) * P],
                                        woT[:, dc, oc2 * 512 : (oc2 + 1) * 512],
                                        start=(dc == 0),
                                        stop=(dc == 7),
                                    )
                                nc.vector.tensor_tensor(
                                    ot[:, oc2 * 512 : (oc2 + 1) * 512],
                                    ops_[:],
                                    bo_bc[:, oc2 * 512 : (oc2 + 1) * 512],
                                    ALU.add,
                                )
                            nc.sync.dma_start(out[ic * P : (ic + 1) * P, :], ot[:])

    nc.compile()
    return nc


def _get_nc():
    if "nc" not in _CACHE:
        _CACHE["nc"] = build()
    return _CACHE["nc"]


def build_in_maps(inputs):
    values = np.ascontiguousarray(inputs["values"], dtype=np.float32)
    keys = np.ascontiguousarray(inputs["keys"], dtype=np.float32)
    query = np.ascontiguousarray(inputs["query"], dtype=np.float32)
    Wv = np.ascontiguousarray(inputs["Wv"], dtype=np.float32)
    Wk = np.ascontiguousarray(inputs["Wk"], dtype=np.float32)
    Wq = np.ascontiguousarray(inputs["Wq"], dtype=np.float32)
    Wo = np.ascontiguousarray(inputs["Wo"], dtype=np.float32)
    bo_ = np.ascontiguousarray(inputs["bo"], dtype=np.float32).reshape(1, D)
    ident = np.eye(P, dtype=np.float32)
    ones = np.ones((P, 2 * H), dtype=np.float16)
    in_maps = []
    for c in range(8):
        b, half = c // 2, c % 2
        in_maps.append(
            {
                "xq": np.ascontiguousarray(
                    query[b, half * NQ : (half + 1) * NQ, :]
                ),
                "xk": keys[b],
                "xv": values[b],
                "wq": Wq,
                "wk": Wk,
                "wv": Wv,
                "wo": Wo,
                "bo": bo_,
                "ident": ident,
                "ones": ones,
            }
        )
    return in_maps


def kernel(values, keys, query, Wv, Wk, Wq, Wo, bo):
    inputs = {
        "values": values, "keys": keys, "query": query,
        "Wv": Wv, "Wk": Wk, "Wq": Wq, "Wo": Wo, "bo": bo,
    }
    in_maps = build_in_maps(inputs)
    nc = _get_nc()
    res = run_bass_kernel_spmd(nc, in_maps, core_ids=list(range(8)))

    B, S = 4, 2048
    out = np.empty((B, S, D), dtype=np.float32)
    for c in range(8):
        b, half = c // 2, c % 2
        out[b, half * NQ : (half + 1) * NQ, :] = res.results[c]["out"]
    return out



# revision 10
# speedup vs baseline: 1.4202x; 1.4202x over previous
"""Multi-head attention (nn_AttentionMechanism) on 8 Trainium2 NeuronCores.

Reference computation (per batch n):
    v = values @ Wv.T ; k = keys @ Wk.T ; q = query @ Wq.T   (all [S, D])
    energy[h,i,j] = sum_d q[i,h,d] k[j,h,d]
    attn = softmax(energy / sqrt(D), axis=j)
    out = (attn @ v per head, concat heads) @ Wo.T + bo

Sharding: data-parallel over (batch, seq-half): core c handles batch c//2,
query rows (c%2)*1024..+1024. K/V are computed for the full 2048-row sequence
on both cores of a pair (duplicated compute hides under the ScalarE exp
stream, zero collectives).

v2 design (vs the v1 baseline):
 - All transposes + fp32->fp16 casts moved to the HOST: inputs arrive as
   pre-transposed fp16 (wqT/wkT/wvT/woT [in,out], xqT/xkT/xvT [d, row]).
   This deletes ~700 PE-transpose instructions and halves input DMA bytes.
 - The softmax 1/sqrt(D) scale is folded into wqT on the host, so the exp
   ACTIVATE runs with scale=1.
 - V is projected DIRECTLY into the attn@v operand layout (vx[k, head, dim])
   by swapping matmul operands: stationary = xvT chunk, moving = wvT columns
   for 4 head-pairs at once (N=512). No v re-transpose pass.
 - Same phase-B attention dataflow as v1: energy in [k, q] orientation, exp
   on ScalarE, attn@v with a ones-column carrying the softmax denominator.
 - Softmax without max-subtraction (energy/32 is ~N(0, 0.25); exp never
   overflows for this problem's input distribution).
"""

import numpy as np

import concourse.bass as bass
import concourse.mybir as mybir
import concourse.tile as tile
from concourse import bacc
from concourse.bass_utils import run_bass_kernel_spmd

F32 = mybir.dt.float32
F16 = mybir.dt.float16
AF = mybir.ActivationFunctionType
ALU = mybir.AluOpType

P = 128
D = 1024
H = 16
DH = 64
NQ = 1024  # q rows per core
NK = 2048  # kv rows per core

_CACHE = {}


def build():
    nc = bacc.Bacc("TRN2", target_bir_lowering=False, debug=False)

    # all pre-transposed fp16 (host-side prep); wqT carries the 1/32 scale
    xqT_d = nc.dram_tensor("xqT", [D, NQ], F16, kind="ExternalInput")
    xkT_d = nc.dram_tensor("xkT", [D, NK], F16, kind="ExternalInput")
    xvT_d = nc.dram_tensor("xvT", [D, NK], F16, kind="ExternalInput")
    wqT_d = nc.dram_tensor("wqT", [D, D], F16, kind="ExternalInput")
    wkT_d = nc.dram_tensor("wkT", [D, D], F16, kind="ExternalInput")
    wvT_d = nc.dram_tensor("wvT", [D, D], F16, kind="ExternalInput")
    woT_d = nc.dram_tensor("woT", [D, D], F16, kind="ExternalInput")
    bo_d = nc.dram_tensor("bo", [1, D], F32, kind="ExternalInput")
    out = nc.dram_tensor("out", [NQ, D], F32, kind="ExternalOutput")

    def load_T(dst, src, n):
        """dst[p, dc, :] = src[dc*128+p, :]  (n = free size)"""
        nc.sync.dma_start(dst[:], src[:].rearrange("(dc p) n -> p dc n", p=P))

    with tile.TileContext(nc) as tc:
        with (
            tc.tile_pool(name="glob", bufs=1) as glob,
            tc.tile_pool(name="bglob", bufs=1) as bglob,
            tc.tile_pool(name="wt", bufs=1) as wtp,
            tc.tile_pool(name="vxg", bufs=1) as vxg,
        ):
            qT = glob.tile([P, 8, NQ], F16, name="qT")      # 16 KB/part
            catT = glob.tile([P, 8, NQ], F16, name="catT")  # 16 KB/part
            xkT = bglob.tile([P, 8, NK], F16, name="xkT")   # 32 KB
            xvT = bglob.tile([P, 8, NK], F16, name="xvT")   # 32 KB
            wkT = wtp.tile([P, 8, D], F16, name="wkT")      # 16 KB
            wvT = wtp.tile([P, 8, D], F16, name="wvT")      # 16 KB
            # vx[p, pair, kc, head, 0:64] = v value; [.., 64] = 1.0
            vx = vxg.tile([P, 8, 16, 2, 65], F16, name="vx")  # 33 KB

            # ---------------- Phase A ----------------
            # load everything; q projection; vx for pairs 0-3
            with (
                tc.tile_pool(name="aq", bufs=1) as aqp,
                tc.tile_pool(name="psA", bufs=1, space="PSUM") as psA,
            ):
                wqT = aqp.tile([P, 8, D], F16, name="wqT")
                xqT = aqp.tile([P, 8, NQ], F16, name="xqT")
                load_T(wqT, wqT_d, D)
                load_T(xqT, xqT_d, NQ)
                load_T(wkT, wkT_d, D)
                load_T(xkT, xkT_d, NK)
                load_T(wvT, wvT_d, D)
                load_T(xvT, xvT_d, NK)
                # ones columns for all pairs (engine write, not DMA — a
                # 2-byte-strided DMA scatter read-modify-writes granules
                # that the vx value-copies write concurrently)
                nc.gpsimd.memset(vx[:, :, :, :, 64:65], 1.0)

                # q projection, pair-major so pair 0 is ready first
                for c in range(8):
                    for ib in range(2):
                        qps = psA.tile(
                            [P, 512], F32, tag="qps", bufs=2,
                            name=f"qps{c}_{ib}",
                        )
                        for dc in range(8):
                            nc.tensor.matmul(
                                qps[:],
                                wqT[:, dc, c * P : (c + 1) * P],
                                xqT[:, dc, ib * 512 : (ib + 1) * 512],
                                start=(dc == 0),
                                stop=(dc == 7),
                            )
                        nc.scalar.copy(
                            qT[:, c, ib * 512 : (ib + 1) * 512], qps[:]
                        )

                # vx for pairs 0-3 (N=512 over 4 pairs' channels)
                for k2 in range(16):
                    vps = psA.tile(
                        [P, 512], F32, tag="vps", bufs=2, name=f"vpsA{k2}"
                    )
                    for dc in range(8):
                        nc.tensor.matmul(
                            vps[:],
                            xvT[:, dc, k2 * P : (k2 + 1) * P],
                            wvT[:, dc, 0:512],
                            start=(dc == 0),
                            stop=(dc == 7),
                        )
                    nc.vector.tensor_copy(
                        vx[:, 0:4, k2, :, 0:64],
                        vps[:].rearrange("p (c t d) -> p c t d", t=2, d=DH),
                    )

            # ---------------- Phase B ----------------
            with tc.tile_pool(name="wop", bufs=1) as wop:
                woT = wop.tile([P, 8, D], F16, name="woT")  # 16 KB
                load_T(woT, woT_d, D)

                with (
                    tc.tile_pool(name="kv", bufs=2) as kvp,
                    tc.tile_pool(name="pp", bufs=3) as ppp,
                    tc.tile_pool(name="dd", bufs=2) as ddp,
                    tc.tile_pool(name="psB", bufs=1, space="PSUM") as psB,
                ):

                    def make_preamble(c):
                        """Allocate pair-c kT; return (kT, steps).

                        Each step is a thunk emitting one chunk of the k
                        projection (and, for early pairs, part of the
                        vx build for pairs 4-7) so it can be interleaved
                        into the previous pair's attention.
                        """
                        kT = kvp.tile([P, NK], F16, tag="kt", name=f"kT{c}")
                        steps = []

                        def kproj_step(ic4):
                            def _f():
                                ps_ = psB.tile(
                                    [P, 512], F32, tag="kvps", bufs=2,
                                    name=f"kps{c}_{ic4}",
                                )
                                for dc in range(8):
                                    nc.tensor.matmul(
                                        ps_[:],
                                        wkT[:, dc, c * P : (c + 1) * P],
                                        xkT[:, dc, ic4 * 512 : (ic4 + 1) * 512],
                                        start=(dc == 0),
                                        stop=(dc == 7),
                                    )
                                nc.vector.tensor_copy(
                                    kT[:, ic4 * 512 : (ic4 + 1) * 512],
                                    ps_[:],
                                )

                            return _f

                        for ic4 in range(4):
                            steps.append(kproj_step(ic4))

                        # vx for pairs 4-7: 16 k2-chunks spread over the
                        # preambles of pairs 1..3 (done before pair 4 runs);
                        # each k2 split into two steps (psum-accumulate
                        # halves) to keep interleave granularity fine.
                        if 1 <= c <= 3:
                            vps_box = {}

                            def vx47_mm(k2, half):
                                def _f():
                                    if half == 0:
                                        vps_box[k2] = psB.tile(
                                            [P, 512], F32, tag="kvps", bufs=2,
                                            name=f"vps{k2}",
                                        )
                                    ps_ = vps_box[k2]
                                    for dc in range(4 * half, 4 * half + 4):
                                        nc.tensor.matmul(
                                            ps_[:],
                                            xvT[:, dc, k2 * P : (k2 + 1) * P],
                                            wvT[:, dc, 512:1024],
                                            start=(dc == 0),
                                            stop=(dc == 7),
                                        )
                                    if half == 1:
                                        nc.vector.tensor_copy(
                                            vx[:, 4:8, k2, :, 0:64],
                                            ps_[:].rearrange(
                                                "p (cc t d) -> p cc t d",
                                                t=2, d=DH,
                                            ),
                                        )
                                        del vps_box[k2]

                                return _f

                            # pair c=1 -> k2 0..5, c=2 -> 6..11, c=3 -> 12..15
                            k2s = {
                                1: range(0, 6),
                                2: range(6, 12),
                                3: range(12, 16),
                            }[c]
                            for k2 in k2s:
                                steps.append(vx47_mm(k2, 0))
                                steps.append(vx47_mm(k2, 1))

                        return kT, steps

                    # prologue: pair 0's kproj runs un-overlapped
                    kT, steps = make_preamble(0)
                    for st in steps:
                        st()

                    for c in range(8):  # head pair
                        if c < 7:
                            kT_n, steps = make_preamble(c + 1)
                        else:
                            kT_n, steps = None, []
                        si = 0
                        for qt in range(2):
                            o0 = psB.tile(
                                [65, 512], F32, tag="o0", bufs=1,
                                name=f"o0_{c}_{qt}",
                            )
                            o1 = psB.tile(
                                [65, 512], F32, tag="o1", bufs=1,
                                name=f"o1_{c}_{qt}",
                            )

                            def energy(kc):
                                ee = psB.tile(
                                    [P, 1024], F32, tag="ee", bufs=2,
                                    name=f"ee_{c}_{qt}_{kc}",
                                )
                                nc.tensor.matmul(
                                    ee[:, 0:512],
                                    kT[0:DH, kc * P : (kc + 1) * P],
                                    qT[0:DH, c, qt * 512 : (qt + 1) * 512],
                                    start=True,
                                    stop=True,
                                )
                                nc.tensor.matmul(
                                    ee[:, 512:1024],
                                    kT[DH:P, kc * P : (kc + 1) * P],
                                    qT[DH:P, c, qt * 512 : (qt + 1) * 512],
                                    start=True,
                                    stop=True,
                                )
                                pp = ppp.tile(
                                    [P, 1024], F16, tag="pp",
                                    name=f"pp_{c}_{qt}_{kc}",
                                )
                                nc.scalar.activation(pp[:], ee[:], AF.Exp)
                                return pp

                            # energy runs one iteration ahead of attn@v
                            # so the in-order PE stream never stalls on
                            # the exp of the current iteration.
                            pp_cur = energy(0)
                            for kc in range(16):
                                if kc < 15:
                                    pp_nxt = energy(kc + 1)
                                nc.tensor.matmul(
                                    o0[:],
                                    vx[:, c, kc, 0, :],
                                    pp_cur[:, 0:512],
                                    start=(kc == 0),
                                    stop=(kc == 15),
                                )
                                nc.tensor.matmul(
                                    o1[:],
                                    vx[:, c, kc, 1, :],
                                    pp_cur[:, 512:1024],
                                    start=(kc == 0),
                                    stop=(kc == 15),
                                )
                                if kc < 15:
                                    pp_cur = pp_nxt
                                # interleave one next-pair preamble step
                                # every other iteration
                                if kc % 2 == 1 and si < len(steps):
                                    steps[si]()
                                    si += 1
                            # normalize: catT[rows, c, qt] = o[0:64]/o[64]
                            for j, ops in enumerate((o0, o1)):
                                stage = ddp.tile(
                                    [P, 512], F32, tag="stage",
                                    name=f"stage{c}_{qt}_{j}",
                                )
                                nc.vector.tensor_copy(
                                    stage[0:65, :], ops[0:65, :]
                                )
                                dsh = ddp.tile(
                                    [1, 512], F32, tag="dsh",
                                    name=f"dsh{c}_{qt}_{j}",
                                )
                                nc.sync.dma_start(
                                    dsh[0:1, :], stage[64:65, :]
                                )
                                rec = ddp.tile(
                                    [1, 512], F32, tag="rec", bufs=1,
                                    name=f"rec{c}_{qt}_{j}",
                                )
                                nc.vector.reciprocal_approx_fast(
                                    out=rec[0:1, :], in_=dsh[0:1, :]
                                )
                                bc = ddp.tile(
                                    [DH, 512], F32, tag="bc",
                                    name=f"bc{c}_{qt}_{j}",
                                )
                                nc.gpsimd.partition_broadcast(
                                    bc[:], rec[0:1, :]
                                )
                                if j == 0:
                                    nc.vector.tensor_tensor(
                                        catT[
                                            0:DH, c, qt * 512 : (qt + 1) * 512
                                        ],
                                        stage[0:DH, :],
                                        bc[:],
                                        ALU.mult,
                                    )
                                else:
                                    stg = ddp.tile(
                                        [DH, 512], F16, tag="stg",
                                        name=f"stg{c}_{qt}",
                                    )
                                    nc.vector.tensor_tensor(
                                        stg[:], stage[0:DH, :], bc[:],
                                        ALU.mult,
                                    )
                                    nc.sync.dma_start(
                                        catT[
                                            DH:P, c, qt * 512 : (qt + 1) * 512
                                        ],
                                        stg[:],
                                    )
                        # any remaining preamble steps
                        while si < len(steps):
                            steps[si]()
                            si += 1
                        kT = kT_n

                # ---------------- Phase C: output projection ----------
                with (
                    tc.tile_pool(name="osb", bufs=3) as osbp,
                    tc.tile_pool(name="psC", bufs=1, space="PSUM") as psC,
                ):
                    bo_st = osbp.tile(
                        [P, D], F32, tag="bo_st", bufs=1, name="bo_st"
                    )
                    nc.sync.dma_start(bo_st[0:1, :], bo_d[:])
                    bo_bc = osbp.tile(
                        [P, D], F32, tag="bo_bc", bufs=1, name="bo_bc"
                    )
                    nc.gpsimd.partition_broadcast(bo_bc[:], bo_st[0:1, :])

                    for ic in range(8):
                        ot = osbp.tile(
                            [P, D], F32, tag="ot", bufs=3, name=f"ot{ic}"
                        )
                        for oc2 in range(2):
                            ops_ = psC.tile(
                                [P, 512], F32, tag="ops", bufs=2,
                                name=f"ops{ic}_{oc2}",
                            )
                            for dc in range(8):
                                nc.tensor.matmul(
                                    ops_[:],
                                    catT[:, dc, ic * P : (ic + 1) * P],
                                    woT[:, dc, oc2 * 512 : (oc2 + 1) * 512],
                                    start=(dc == 0),
                                    stop=(dc == 7),
                                )
                            nc.vector.tensor_tensor(
                                ot[:, oc2 * 512 : (oc2 + 1) * 512],
                                ops_[:],
                                bo_bc[:, oc2 * 512 : (oc2 + 1) * 512],
                                ALU.add,
                            )
                        nc.sync.dma_start(out[ic * P : (ic + 1) * P, :], ot[:])

    nc.compile()
    return nc


def _get_nc():
    if "nc" not in _CACHE:
        _CACHE["nc"] = build()
    return _CACHE["nc"]


def build_in_maps(inputs):
    f16 = np.float16
    values = np.asarray(inputs["values"], dtype=np.float32)
    keys = np.asarray(inputs["keys"], dtype=np.float32)
    query = np.asarray(inputs["query"], dtype=np.float32)
    # pre-transposed fp16 weights; softmax scale folded into wqT
    wqT = np.ascontiguousarray(
        (np.asarray(inputs["Wq"], dtype=np.float32).T / 32.0).astype(f16)
    )
    wkT = np.ascontiguousarray(
        np.asarray(inputs["Wk"], dtype=np.float32).T.astype(f16)
    )
    wvT = np.ascontiguousarray(
        np.asarray(inputs["Wv"], dtype=np.float32).T.astype(f16)
    )
    woT = np.ascontiguousarray(
        np.asarray(inputs["Wo"], dtype=np.float32).T.astype(f16)
    )
    bo_ = np.ascontiguousarray(inputs["bo"], dtype=np.float32).reshape(1, D)
    ones = np.ones((P, 2 * H), dtype=f16)
    in_maps = []
    for c in range(8):
        b, half = c // 2, c % 2
        in_maps.append(
            {
                "xqT": np.ascontiguousarray(
                    query[b, half * NQ : (half + 1) * NQ, :].T.astype(f16)
                ),
                "xkT": np.ascontiguousarray(keys[b].T.astype(f16)),
                "xvT": np.ascontiguousarray(values[b].T.astype(f16)),
                "wqT": wqT,
                "wkT": wkT,
                "wvT": wvT,
                "woT": woT,
                "bo": bo_,
            }
        )
    return in_maps


def kernel(values, keys, query, Wv, Wk, Wq, Wo, bo):
    inputs = {
        "values": values, "keys": keys, "query": query,
        "Wv": Wv, "Wk": Wk, "Wq": Wq, "Wo": Wo, "bo": bo,
    }
    in_maps = build_in_maps(inputs)
    nc = _get_nc()
    res = run_bass_kernel_spmd(nc, in_maps, core_ids=list(range(8)))

    B, S = 4, 2048
    out = np.empty((B, S, D), dtype=np.float32)
    for c in range(8):
        b, half = c // 2, c % 2
        out[b, half * NQ : (half + 1) * NQ, :] = res.results[c]["out"]
    return out
